# revision 7
# baseline (speedup 1.0000x reference)
"""Trainium2 Bass kernel for nn_LinearLLM: out[b,t,v] = sum_{s>=t,w} x[b,s,w]*W[s,w,t,v] + bias.

Contraction axis s is sharded across the 8 cores (cyclic over 128-row
K-chunks = 2 s-values x 64 w); each core computes partial sums for all
3078 = 513*6 (t,v) columns and the 8 bf16 partials are summed on host.

Weights are fp8 e3m4 (x2048) quantized with greedy error-feedback
rounding that cancels quantization error inside the 6-dim subspace
spanned by the (also e3m4) embedding rows — end-to-end rel err ~3e-3.

Raw bass (no TileContext): hand-rolled semaphores avoid the Tile
scheduler's extra barrier/teardown overhead, and the input stream is
split over TWO HWDGE rings (SP + Activation issue queues) so descriptor
handoff gaps on one ring are covered by the other; a single ring was
measured at ~255 B/ns, below the ~307 B/ns the PE consumes.

Per core: 32 K-chunk positions in DESCENDING s order (position i has
uniform width 96*(32-i), zero-padded per core, so one SPMD program) +
a 1/8 column slice of the final s=512 chunk (K=64). PSUM banks 0-5
accumulate; as s descends, high-t banks stop receiving contributions
and are drained (DVE cast to bf16, out-DMA on the SP ring) while
compute continues.
"""
import numpy as np
import ml_dtypes
from contextlib import ExitStack

from concourse import bacc
from concourse.bass_utils import run_bass_kernel_spmd
import concourse.mybir as mybir

B, L1, EMB, V, NCORES = 128, 513, 64, 6, 8
NPOS = 32
SCALE = 2048.0
NCOLS = 3072
XCOLS = 385
OUTC = NCOLS + XCOLS
W_DT = mybir.dt.float8e3
NP_W = ml_dtypes.float8_e3m4

WIDTHS = [96 * (NPOS - i) for i in range(NPOS)]          # 3072 ... 96
XT_COLS = (NPOS + 1) * 128                               # 4224

# position groups alternate between ring A (SP) and ring B (Activation)
# in position order so delivery order matches PE need order
GROUPS = [[0], [1], [2], [3], [4, 5], [6, 7], [8, 9, 10], [11, 12, 13],
          [14, 15, 16, 17], [18, 19, 20, 21, 22],
          [23, 24, 25, 26, 27, 28, 29, 30, 31]]
SLAB_MAIN = 128 * sum(WIDTHS)
SLAB_TOTAL = SLAB_MAIN + 64 * XCOLS
I_STOP = [max(i for i in range(NPOS) if WIDTHS[i] > 512 * j) for j in range(6)]

_CACHE = {}


def _build():
    if "nc" in _CACHE:
        return _CACHE["nc"]
    nc = bacc.Bacc("TRN2", target_bir_lowering=False, debug=False,
                   num_devices=NCORES)
    xt_dram = nc.declare_dram_parameter("xt", [128, XT_COLS], W_DT,
                                        isOutput=False)
    slab_dram = nc.declare_dram_parameter("slab", [SLAB_TOTAL], W_DT,
                                          isOutput=False)
    out_dram = nc.declare_dram_parameter("out", [128, OUTC],
                                         mybir.dt.bfloat16, isOutput=True)

    goffs = []
    off = 0
    for g in GROUPS:
        goffs.append(off)
        off += 128 * sum(WIDTHS[i] for i in g)
    assert off == SLAB_MAIN

    def slab_ap(off, n, p=128):
        return slab_dram[off:off + p * n].rearrange("(p n) -> p n", p=p)

    with ExitStack() as es:
        xt = es.enter_context(nc.sbuf_tensor("xt_sb", [128, XT_COLS], W_DT))
        wgs = [es.enter_context(
            nc.sbuf_tensor(f"wg{gi}", [128, sum(WIDTHS[i] for i in g)], W_DT))
            for gi, g in enumerate(GROUPS)]
        w32 = es.enter_context(nc.sbuf_tensor("w32_sb", [64, XCOLS], W_DT))
        obufs = [es.enter_context(
            nc.sbuf_tensor(f"ob{j}", [128, 512], mybir.dt.bfloat16))
            for j in range(6)]
        ox = es.enter_context(
            nc.sbuf_tensor("ox_sb", [128, XCOLS], mybir.dt.bfloat16))
        pss = [es.enter_context(
            nc.psum_tensor(f"ps{j}", [128, 512], mybir.dt.float32))
            for j in range(6)]
        psx = es.enter_context(
            nc.psum_tensor("psx", [128, XCOLS], mybir.dt.float32))

        S_wa = nc.alloc_semaphore("S_wa")    # ring A (SP) input completions
        S_wb = nc.alloc_semaphore("S_wb")    # ring B (Act) input completions
        S_pe = nc.alloc_semaphore("S_pe")    # PE bank-finished milestones
        S_cp = nc.alloc_semaphore("S_cp")    # DVE drain copies done
        S_out = nc.alloc_semaphore("S_out")  # out DMA completions

        # ---- input DMA rings: A = SP issue queue, B = Activation's ----
        ring_a = [0, 2, 4, 6, 8, 10]
        ring_b = ["xt", 1, 3, "w32", 5, 7, 9]

        def issue_ring(eng, items, sem):
            thresh, v = {}, 0
            for gi in items:
                v += 16
                if gi == "xt":
                    eng.dma_start(xt[:], xt_dram[:]).then_inc(sem, 16)
                elif gi == "w32":
                    eng.dma_start(
                        w32[:], slab_ap(SLAB_MAIN, XCOLS, p=64)
                    ).then_inc(sem, 16)
                else:
                    wsum = sum(WIDTHS[i] for i in GROUPS[gi])
                    eng.dma_start(
                        wgs[gi][:], slab_ap(goffs[gi], wsum)).then_inc(sem, 16)
                thresh[gi] = v
            return thresh

        wa_thresh = issue_ring(nc.sync, ring_a, S_wa)
        wb_thresh = issue_ring(nc.scalar, ring_b, S_wb)

        # ---- PE: positions descending in s; drains signal S_pe ----
        pos_group = {}
        pos_off = {}
        for gi, g in enumerate(GROUPS):
            o = 0
            for i in g:
                pos_group[i] = gi
                pos_off[i] = o
                o += WIDTHS[i]

        pe_milestone = 0
        for i in range(NPOS):
            gi = pos_group[i]
            if pos_off[i] == 0:              # first position of its group
                if gi % 2 == 0:
                    nc.tensor.wait_ge(S_wa, wa_thresh[gi])
                else:
                    nc.tensor.wait_ge(S_wb, wb_thresh[gi])
                if i == 0:
                    nc.tensor.wait_ge(S_wb, wb_thresh["xt"])
            w_i = WIDTHS[i]
            for j in range((w_i + 511) // 512):
                c0, c1 = 512 * j, min(512 * (j + 1), w_i)
                mm = nc.tensor.matmul(
                    pss[j][:, :c1 - c0],
                    xt[:, 128 * i:128 * (i + 1)],
                    wgs[gi][:, pos_off[i] + c0:pos_off[i] + c1],
                    start=(i == 0),
                    stop=(i == I_STOP[j]),
                )
                if i == I_STOP[j]:
                    pe_milestone += 1
                    mm.then_inc(S_pe, 1)
            if i == 8:                       # s=512 chunk, own bank
                nc.tensor.wait_ge(S_wb, wb_thresh["w32"])
                pe_milestone += 1
                nc.tensor.matmul(psx[:],
                                 xt[0:64, 128 * NPOS:128 * (NPOS + 1)],
                                 w32[0:64, :],
                                 start=True, stop=True).then_inc(S_pe, 1)

        # ---- DVE: drain copies in PE milestone order ----
        # milestones: pos5->bank5 (1), pos8->psx (2), pos10->bank4 (3),
        # pos15->bank3 (4), pos21->bank2 (5), pos26->bank1 (6), pos31->bank0 (7)
        drains = [(5, pss[5], obufs[5], 512, 2560),
                  (-1, psx, ox, XCOLS, NCOLS),
                  (4, pss[4], obufs[4], 512, 2048),
                  (3, pss[3], obufs[3], 512, 1536),
                  (2, pss[2], obufs[2], 512, 1024),
                  (1, pss[1], obufs[1], 512, 512),
                  (0, pss[0], obufs[0], 512, 0)]
        for n, (jb, ps, ob, cols, dst0) in enumerate(drains):
            nc.vector.wait_ge(S_pe, n + 1)
            nc.vector.tensor_copy(ob[:, :cols], ps[:, :cols]).then_inc(S_cp, 1)

        # ---- SP: out DMAs after its input issues; final completion wait ----
        for n, (jb, ps, ob, cols, dst0) in enumerate(drains):
            nc.sync.wait_ge(S_cp, n + 1)
            nc.sync.dma_start(out_dram[:, dst0:dst0 + cols],
                              ob[:, :cols]).then_inc(S_out, 16)
        nc.sync.wait_ge(S_out, 16 * len(drains))

    nc.compile()
    _CACHE["nc"] = nc
    return nc


def _quantize_weights(emb, W):
    """Greedy error-feedback e3m4 quantization of SCALE*W.

    Returns (xq8 (6,64) e3m4, Wq8 (513,513,6,64) e3m4 scaled, masked t<=s).
    Rounding of each 64-element w-row chooses floor/ceil per element to
    cancel the running residual r = A(q-w) + b0 where A = dequantized
    e3m4 embedding and b0 compensates the embedding's own quant error.
    """
    emb = np.asarray(emb, np.float32)
    W = np.asarray(W, np.float32)
    xq8 = emb.astype(NP_W)
    xq = xq8.astype(np.float32)                     # (6,64)
    ex = xq - emb

    Ws = W * SCALE                                  # (513,64,513,6) fp32
    B0 = np.tensordot(ex, Ws, axes=([1], [1]))      # (6,513,513,6)

    Wr = np.ascontiguousarray(Ws.transpose(0, 2, 3, 1)).reshape(-1, EMB)
    del Ws
    s_idx = np.repeat(np.arange(L1), L1 * V)
    t_idx = np.tile(np.repeat(np.arange(L1), V), L1)
    valid = t_idx <= s_idx
    Wv = np.ascontiguousarray(Wr[valid])            # (Nv, 64)
    r = np.ascontiguousarray(
        B0.transpose(1, 2, 3, 0).reshape(-1, V)[valid])
    del B0

    allb = np.arange(256, dtype=np.uint8)
    vals = allb.view(NP_W).astype(np.float32)
    grid = np.unique(vals[np.isfinite(vals)])
    lo_i = np.searchsorted(grid, Wv, side="right") - 1
    lo = grid[np.clip(lo_i, 0, len(grid) - 1)]
    hi = grid[np.clip(lo_i + 1, 0, len(grid) - 1)]
    del lo_i
    Q = np.empty_like(Wv)

    A = xq.T.copy()                                 # (64, 6)
    order = np.argsort(-np.linalg.norm(A, axis=1))
    for j in order:
        aj = A[j]
        n2 = float(aj @ aj)
        g = r @ aj
        dlo = lo[:, j] - Wv[:, j]
        dhi = hi[:, j] - Wv[:, j]
        pick_hi = 2 * g * dhi + dhi * dhi * n2 < 2 * g * dlo + dlo * dlo * n2
        d = np.where(pick_hi, dhi, dlo)
        Q[:, j] = np.where(pick_hi, hi[:, j], lo[:, j])
        r += d[:, None] * aj
    for j in order:                                 # one refinement sweep
        aj = A[j]
        n2 = float(aj @ aj)
        g = r @ aj
        cur = Q[:, j]
        other = np.where(cur == lo[:, j], hi[:, j], lo[:, j])
        dd = other - cur
        flip = 2 * g * dd + dd * dd * n2 < 0
        Q[:, j] = np.where(flip, other, cur)
        r += np.where(flip, dd, 0.0)[:, None] * aj

    Wq = np.zeros_like(Wr)
    Wq[valid] = Q
    Wq8 = Wq.reshape(L1, L1, V, EMB).astype(NP_W)   # (s,t,v,w)
    return xq8, Wq8


def _prep_inputs(src, embedding, weight):
    src = np.asarray(src)
    xq8, Wq8 = _quantize_weights(embedding, weight)

    xfull = xq8[src]                                # (B, 513, 64) e3m4
    row512 = np.ascontiguousarray(
        Wq8[512].transpose(2, 0, 1)).reshape(EMB, L1 * V)   # (64, 3078)

    in_maps = []
    for c in range(NCORES):
        ks = [8 * (NPOS - 1 - i) + c for i in range(NPOS)]
        s_arr = np.array([[2 * k, 2 * k + 1] for k in ks])   # (32,2)
        sel = xfull[:, s_arr, :]                     # (B,32,2,64)
        xt = np.zeros((128, NPOS + 1, 128), NP_W)
        xt[:, :NPOS, :] = sel.transpose(2, 3, 1, 0).reshape(128, NPOS, B)
        xt[:EMB, NPOS, :] = xfull[:, 512, :].T
        xt2 = np.ascontiguousarray(xt.reshape(128, XT_COLS))

        parts = []
        for g in GROUPS:
            blks = []
            for i in g:
                k = ks[i]
                t_hi = WIDTHS[i] // V
                arr = Wq8[2 * k:2 * k + 2, :t_hi, :, :]     # (2,t_hi,6,64)
                blks.append(arr.transpose(0, 3, 1, 2).reshape(128, WIDTHS[i]))
            parts.append(np.ascontiguousarray(
                np.concatenate(blks, axis=1)).reshape(-1))
        w32 = np.zeros((64, XCOLS), NP_W)
        c0 = XCOLS * c
        c1 = min(c0 + XCOLS, L1 * V)
        w32[:, :c1 - c0] = row512[:, c0:c1]
        parts.append(w32.reshape(-1))
        slab = np.concatenate(parts)
        assert slab.shape[0] == SLAB_TOTAL
        in_maps.append({"xt": xt2, "slab": slab})
    return in_maps


def _unshard(results, bias):
    full = np.zeros((B, L1 * V), np.float32)
    for c in range(NCORES):
        o = results[c]["out"].astype(np.float32)
        full[:, :NCOLS] += o[:, :NCOLS]
        c0 = XCOLS * c
        c1 = min(c0 + XCOLS, L1 * V)
        full[:, c0:c1] += o[:, NCOLS:NCOLS + (c1 - c0)]
    full *= 1.0 / SCALE
    full = full.reshape(B, L1, V) + np.asarray(bias, np.float32)[None]
    return np.ascontiguousarray(full.transpose(0, 2, 1))


def kernel(src, embedding, weight, bias):
    nc = _build()
    in_maps = _prep_inputs(src, embedding, weight)
    res = run_bass_kernel_spmd(nc, in_maps, list(range(NCORES)))
    return _unshard(res.results, bias)


# revision 16
# speedup vs baseline: 1.0521x; 1.0521x over previous
"""Trainium2 Bass kernel for nn_LinearLLM: out[b,t,v] = sum_{s>=t,w} x[b,s,w]*W[s,w,t,v] + bias.

Contraction axis s is sharded across the 8 cores (cyclic over 128-row
K-chunks = 2 s-values x 64 w); each core computes partial sums for all
3078 = 513*6 (t,v) columns and the 8 bf16 partials are summed on host.

Weights are fp8 e3m4 (x2048) quantized with greedy error-feedback
rounding that cancels quantization error inside the 6-dim subspace
spanned by the (also e3m4) embedding rows — end-to-end rel err ~3e-3.

Raw bass (no TileContext): hand-rolled semaphores avoid the Tile
scheduler's extra barrier/teardown overhead, and the input stream is
split over TWO HWDGE rings (SP + Activation issue queues) so descriptor
handoff gaps on one ring are covered by the other; a single ring was
measured at ~255 B/ns, below the ~307 B/ns the PE consumes.

Per core: 32 K-chunk positions in DESCENDING s order (position i has
uniform width 96*(32-i), zero-padded per core, so one SPMD program) +
a 1/8 column slice of the final s=512 chunk (K=64). PSUM banks 0-5
accumulate; as s descends, high-t banks stop receiving contributions
and are drained (DVE cast to bf16, out-DMA on the SP ring) while
compute continues.
"""
import numpy as np
import ml_dtypes
from contextlib import ExitStack

from concourse import bacc
from concourse.bass_utils import run_bass_kernel_spmd
import concourse.mybir as mybir

B, L1, EMB, V, NCORES = 128, 513, 64, 6, 8
NPOS = 32
SCALE = 2048.0
NCOLS = 3072
XCOLS = 385
OUTC = NCOLS + XCOLS
W_DT = mybir.dt.float8e3
NP_W = ml_dtypes.float8_e3m4

WIDTHS = [96 * (NPOS - i) for i in range(NPOS)]          # 3072 ... 96
XT_COLS = (NPOS + 1) * 128                               # 4224

# position groups; the input DMA token sequence (xtA, g0..g7, xtB, w32,
# g8..g12) is split ALTERNATELY across ring A (SP) and ring B (Activation)
# so merged delivery order tracks PE need order at full aggregate rate
GROUPS = [[0], [1], [2], [3], [4], [5], [6], [7], [8, 9, 10], [11, 12, 13],
          [14, 15, 16, 17], [18, 19, 20, 21, 22],
          [23, 24, 25, 26, 27, 28, 29, 30, 31]]
NA = 8                                  # positions in xtA (plus slot for 32)
XT_A_COLS = (NA + 1) * 128              # pos 0..7 + pos 32
XT_B_COLS = (NPOS - NA) * 128           # pos 8..31
SLAB_MAIN = 128 * sum(WIDTHS)
SLAB_TOTAL = SLAB_MAIN + 64 * XCOLS
I_STOP = [max(i for i in range(NPOS) if WIDTHS[i] > 512 * j) for j in range(6)]

_CACHE = {}


def _build():
    if "nc" in _CACHE:
        return _CACHE["nc"]
    nc = bacc.Bacc("TRN2", target_bir_lowering=False, debug=False,
                   num_devices=NCORES)
    xt_dram = nc.declare_dram_parameter("xt", [128, XT_COLS], W_DT,
                                        isOutput=False)
    slab_dram = nc.declare_dram_parameter("slab", [SLAB_TOTAL], W_DT,
                                          isOutput=False)
    out_dram = nc.declare_dram_parameter("out", [128, OUTC],
                                         mybir.dt.bfloat16, isOutput=True)

    goffs = []
    off = 0
    for g in GROUPS:
        goffs.append(off)
        off += 128 * sum(WIDTHS[i] for i in g)
    assert off == SLAB_MAIN

    def slab_ap(off, n, p=128):
        return slab_dram[off:off + p * n].rearrange("(p n) -> p n", p=p)

    with ExitStack() as es:
        xt = es.enter_context(nc.sbuf_tensor("xt_sb", [128, XT_COLS], W_DT))
        wgs = [es.enter_context(
            nc.sbuf_tensor(f"wg{gi}", [128, sum(WIDTHS[i] for i in g)], W_DT))
            for gi, g in enumerate(GROUPS)]
        w32 = es.enter_context(nc.sbuf_tensor("w32_sb", [64, XCOLS], W_DT))
        obufs = [es.enter_context(
            nc.sbuf_tensor(f"ob{j}", [128, 512], mybir.dt.bfloat16))
            for j in range(6)]
        ox = es.enter_context(
            nc.sbuf_tensor("ox_sb", [128, XCOLS], mybir.dt.bfloat16))
        pss = [es.enter_context(
            nc.psum_tensor(f"ps{j}", [128, 512], mybir.dt.float32))
            for j in range(6)]
        psx = es.enter_context(
            nc.psum_tensor("psx", [128, XCOLS], mybir.dt.float32))
        psw = es.enter_context(
            nc.psum_tensor("psw", [128, 512], mybir.dt.float32))

        S_wa = nc.alloc_semaphore("S_wa")    # ring A (SP) input completions
        S_wb = nc.alloc_semaphore("S_wb")    # ring B (Act) input completions
        S_pe = nc.alloc_semaphore("S_pe")    # PE bank-finished milestones
        S_cp = nc.alloc_semaphore("S_cp")    # DVE drain copies done
        S_out = nc.alloc_semaphore("S_out")  # out DMA completions

        # ---- input DMA rings: A = SP issue queue, B = Activation's ----
        tokens = ["xtA", 0, 1, 2, 3, 4, 5, 6, 7, "xtB", "w32", 8, 9, 10,
                  11, 12]
        ring_a = tokens[0::2]
        ring_b = tokens[1::2]

        def issue_ring(eng, items, sem):
            thresh, v = {}, 0
            for gi in items:
                v += 16
                if gi == "xtA":
                    eng.dma_start(xt[:, :XT_A_COLS],
                                  xt_dram[:, :XT_A_COLS]).then_inc(sem, 16)
                elif gi == "xtB":
                    eng.dma_start(xt[:, XT_A_COLS:],
                                  xt_dram[:, XT_A_COLS:]).then_inc(sem, 16)
                elif gi == "w32":
                    eng.dma_start(
                        w32[:], slab_ap(SLAB_MAIN, XCOLS, p=64)
                    ).then_inc(sem, 16)
                else:
                    wsum = sum(WIDTHS[i] for i in GROUPS[gi])
                    eng.dma_start(
                        wgs[gi][:], slab_ap(goffs[gi], wsum)).then_inc(sem, 16)
                thresh[gi] = v
            return thresh

        wa_thresh = issue_ring(nc.sync, ring_a, S_wa)
        wb_thresh = issue_ring(nc.scalar, ring_b, S_wb)
        ring_of = {}
        for t in ring_a:
            ring_of[t] = (S_wa, wa_thresh[t])
        for t in ring_b:
            ring_of[t] = (S_wb, wb_thresh[t])

        # ---- PE: warmup, then positions descending in s ----
        pos_group = {}
        pos_off = {}
        for gi, g in enumerate(GROUPS):
            o = 0
            for i in g:
                pos_group[i] = gi
                pos_off[i] = o
                o += WIDTHS[i]

        def lhsT(i):
            # xt layout: [pos0..7, pos32, pos8..31]
            sl = i if i < NA else (NA if i == NPOS else i + 1)
            return xt[:, 128 * sl:128 * (sl + 1)]

        # (PE p-state warmup via junk matmuls was tried here and produced
        # sporadic NaN on some cores — reverted)

        for i in range(NPOS):
            gi = pos_group[i]
            if pos_off[i] == 0:              # first position of its group
                sem, v = ring_of[gi]
                nc.tensor.wait_ge(sem, v)
                if i == 0:
                    sem, v = ring_of["xtA"]
                    nc.tensor.wait_ge(sem, v)
                if i == 8:
                    sem, v = ring_of["xtB"]
                    nc.tensor.wait_ge(sem, v)
            w_i = WIDTHS[i]
            for j in range((w_i + 511) // 512):
                c0, c1 = 512 * j, min(512 * (j + 1), w_i)
                mm = nc.tensor.matmul(
                    pss[j][:, :c1 - c0],
                    lhsT(i),
                    wgs[gi][:, pos_off[i] + c0:pos_off[i] + c1],
                    start=(i == 0),
                    stop=(i == I_STOP[j]),
                )
                if i == I_STOP[j]:
                    mm.then_inc(S_pe, 1)
            if i == 8:                       # s=512 chunk, own bank
                sem, v = ring_of["w32"]
                nc.tensor.wait_ge(sem, v)
                nc.tensor.matmul(psx[:], xt[0:64, 128 * NA:128 * (NA + 1)],
                                 w32[0:64, :],
                                 start=True, stop=True).then_inc(S_pe, 1)

        # ---- DVE: drain copies in PE milestone order ----
        # milestones: pos5->bank5 (1), pos8->psx (2), pos10->bank4 (3),
        # pos15->bank3 (4), pos21->bank2 (5), pos26->bank1 (6), pos31->bank0 (7)
        drains = [(5, pss[5], obufs[5], 512, 2560),
                  (-1, psx, ox, XCOLS, NCOLS),
                  (4, pss[4], obufs[4], 512, 2048),
                  (3, pss[3], obufs[3], 512, 1536),
                  (2, pss[2], obufs[2], 512, 1024),
                  (1, pss[1], obufs[1], 512, 512),
                  (0, pss[0], obufs[0], 512, 0)]
        for n, (jb, ps, ob, cols, dst0) in enumerate(drains):
            nc.vector.wait_ge(S_pe, n + 1)
            nc.vector.tensor_copy(ob[:, :cols], ps[:, :cols]).then_inc(S_cp, 1)

        # ---- out DMAs: first five on the SP ring, last two (the latency-
        # critical tail) on the Activation ring, idle by then ----
        for n, (jb, ps, ob, cols, dst0) in enumerate(drains):
            eng = nc.sync if n < 5 else nc.scalar
            eng.wait_ge(S_cp, n + 1)
            eng.dma_start(out_dram[:, dst0:dst0 + cols],
                          ob[:, :cols]).then_inc(S_out, 16)
        nc.sync.wait_ge(S_out, 16 * len(drains))

    nc.compile()
    _CACHE["nc"] = nc
    return nc


def _quantize_weights(emb, W):
    """Greedy error-feedback e3m4 quantization of SCALE*W.

    Returns (xq8 (6,64) e3m4, Wq8 (513,513,6,64) e3m4 scaled, masked t<=s).
    Rounding of each 64-element w-row chooses floor/ceil per element to
    cancel the running residual r = A(q-w) + b0 where A = dequantized
    e3m4 embedding and b0 compensates the embedding's own quant error.
    """
    emb = np.asarray(emb, np.float32)
    W = np.asarray(W, np.float32)
    xq8 = emb.astype(NP_W)
    xq = xq8.astype(np.float32)                     # (6,64)
    ex = xq - emb

    Ws = W * SCALE                                  # (513,64,513,6) fp32
    B0 = np.tensordot(ex, Ws, axes=([1], [1]))      # (6,513,513,6)

    Wr = np.ascontiguousarray(Ws.transpose(0, 2, 3, 1)).reshape(-1, EMB)
    del Ws
    s_idx = np.repeat(np.arange(L1), L1 * V)
    t_idx = np.tile(np.repeat(np.arange(L1), V), L1)
    valid = t_idx <= s_idx
    Wv = np.ascontiguousarray(Wr[valid])            # (Nv, 64)
    r = np.ascontiguousarray(
        B0.transpose(1, 2, 3, 0).reshape(-1, V)[valid])
    del B0

    allb = np.arange(256, dtype=np.uint8)
    vals = allb.view(NP_W).astype(np.float32)
    grid = np.unique(vals[np.isfinite(vals)])
    lo_i = np.searchsorted(grid, Wv, side="right") - 1
    lo = grid[np.clip(lo_i, 0, len(grid) - 1)]
    hi = grid[np.clip(lo_i + 1, 0, len(grid) - 1)]
    del lo_i
    Q = np.empty_like(Wv)

    A = xq.T.copy()                                 # (64, 6)
    order = np.argsort(-np.linalg.norm(A, axis=1))
    for j in order:
        aj = A[j]
        n2 = float(aj @ aj)
        g = r @ aj
        dlo = lo[:, j] - Wv[:, j]
        dhi = hi[:, j] - Wv[:, j]
        pick_hi = 2 * g * dhi + dhi * dhi * n2 < 2 * g * dlo + dlo * dlo * n2
        d = np.where(pick_hi, dhi, dlo)
        Q[:, j] = np.where(pick_hi, hi[:, j], lo[:, j])
        r += d[:, None] * aj
    for j in order:                                 # one refinement sweep
        aj = A[j]
        n2 = float(aj @ aj)
        g = r @ aj
        cur = Q[:, j]
        other = np.where(cur == lo[:, j], hi[:, j], lo[:, j])
        dd = other - cur
        flip = 2 * g * dd + dd * dd * n2 < 0
        Q[:, j] = np.where(flip, other, cur)
        r += np.where(flip, dd, 0.0)[:, None] * aj

    Wq = np.zeros_like(Wr)
    Wq[valid] = Q
    Wq8 = Wq.reshape(L1, L1, V, EMB).astype(NP_W)   # (s,t,v,w)
    return xq8, Wq8


def _prep_inputs(src, embedding, weight):
    src = np.asarray(src)
    xq8, Wq8 = _quantize_weights(embedding, weight)

    xfull = xq8[src]                                # (B, 513, 64) e3m4
    row512 = np.ascontiguousarray(
        Wq8[512].transpose(2, 0, 1)).reshape(EMB, L1 * V)   # (64, 3078)

    in_maps = []
    for c in range(NCORES):
        ks = [8 * (NPOS - 1 - i) + c for i in range(NPOS)]
        s_arr = np.array([[2 * k, 2 * k + 1] for k in ks])   # (32,2)
        sel = xfull[:, s_arr, :]                     # (B,32,2,64)
        xtp = sel.transpose(2, 3, 1, 0).reshape(128, NPOS, B)
        xt = np.zeros((128, NPOS + 1, 128), NP_W)
        # slot layout: [pos0..7, pos32, pos8..31]
        xt[:, :NA, :] = xtp[:, :NA, :]
        xt[:EMB, NA, :] = xfull[:, 512, :].T
        xt[:, NA + 1:, :] = xtp[:, NA:, :]
        xt2 = np.ascontiguousarray(xt.reshape(128, XT_COLS))

        parts = []
        for g in GROUPS:
            blks = []
            for i in g:
                k = ks[i]
                t_hi = WIDTHS[i] // V
                arr = Wq8[2 * k:2 * k + 2, :t_hi, :, :]     # (2,t_hi,6,64)
                blks.append(arr.transpose(0, 3, 1, 2).reshape(128, WIDTHS[i]))
            parts.append(np.ascontiguousarray(
                np.concatenate(blks, axis=1)).reshape(-1))
        w32 = np.zeros((64, XCOLS), NP_W)
        c0 = XCOLS * c
        c1 = min(c0 + XCOLS, L1 * V)
        w32[:, :c1 - c0] = row512[:, c0:c1]
        parts.append(w32.reshape(-1))
        slab = np.concatenate(parts)
        assert slab.shape[0] == SLAB_TOTAL
        in_maps.append({"xt": xt2, "slab": slab})
    return in_maps


def _unshard(results, bias):
    full = np.zeros((B, L1 * V), np.float32)
    for c in range(NCORES):
        o = results[c]["out"].astype(np.float32)
        full[:, :NCOLS] += o[:, :NCOLS]
        c0 = XCOLS * c
        c1 = min(c0 + XCOLS, L1 * V)
        full[:, c0:c1] += o[:, NCOLS:NCOLS + (c1 - c0)]
    full *= 1.0 / SCALE
    full = full.reshape(B, L1, V) + np.asarray(bias, np.float32)[None]
    return np.ascontiguousarray(full.transpose(0, 2, 1))


def kernel(src, embedding, weight, bias):
    nc = _build()
    in_maps = _prep_inputs(src, embedding, weight)
    res = run_bass_kernel_spmd(nc, in_maps, list(range(NCORES)))
    return _unshard(res.results, bias)


# revision 17
# speedup vs baseline: 1.0885x; 1.0346x over previous
"""Trainium2 Bass kernel for nn_LinearLLM: out[b,t,v] = sum_{s>=t,w} x[b,s,w]*W[s,w,t,v] + bias.

Contraction axis s is sharded across the 8 cores (cyclic over 128-row
K-chunks = 2 s-values x 64 w); each core computes partial sums for all
3078 = 513*6 (t,v) columns and the 8 bf16 partials are summed on host.

Weights are fp8 e3m4 (x2048) quantized with greedy error-feedback
rounding that cancels quantization error inside the 6-dim subspace
spanned by the (also e3m4) embedding rows — end-to-end rel err ~3e-3.

Raw bass (no TileContext): hand-rolled semaphores avoid the Tile
scheduler's extra barrier/teardown overhead, and the input stream is
split over TWO HWDGE rings (SP + Activation issue queues) so descriptor
handoff gaps on one ring are covered by the other; a single ring was
measured at ~255 B/ns, below the ~307 B/ns the PE consumes.

Per core: 32 K-chunk positions in DESCENDING s order (position i has
uniform width 96*(32-i), zero-padded per core, so one SPMD program) +
a 1/8 column slice of the final s=512 chunk (K=64). PSUM banks 0-5
accumulate; as s descends, high-t banks stop receiving contributions
and are drained (DVE cast to bf16, out-DMA on the SP ring) while
compute continues.
"""
import numpy as np
import ml_dtypes
from contextlib import ExitStack

from concourse import bacc
from concourse.bass_utils import run_bass_kernel_spmd
import concourse.mybir as mybir

B, L1, EMB, V, NCORES = 128, 513, 64, 6, 8
NPOS = 32
SCALE = 2048.0
NCOLS = 3072
XCOLS = 385
OUTC = NCOLS + XCOLS
W_DT = mybir.dt.float8e3
NP_W = ml_dtypes.float8_e3m4

WIDTHS = [96 * (NPOS - i) for i in range(NPOS)]          # 3072 ... 96
XT_COLS = (NPOS + 1) * 128                               # 4224

# position groups; the input DMA token sequence (xtA, g0..g7, xtB, w32,
# g8..g12) is split ALTERNATELY across ring A (SP) and ring B (Activation)
# so merged delivery order tracks PE need order at full aggregate rate
GROUPS = [[0], [1], [2], [3], [4], [5], [6], [7], [8, 9, 10], [11, 12, 13],
          [14, 15, 16, 17], [18, 19, 20, 21, 22],
          [23, 24, 25, 26, 27, 28, 29, 30, 31]]
NA = 8                                  # positions in xtA (plus slot for 32)
XT_A_COLS = (NA + 1) * 128              # pos 0..7 + pos 32
XT_B_COLS = (NPOS - NA) * 128           # pos 8..31
SLAB_MAIN = 128 * sum(WIDTHS)
SLAB_TOTAL = SLAB_MAIN + 64 * XCOLS
I_STOP = [max(i for i in range(NPOS) if WIDTHS[i] > 512 * j) for j in range(6)]

_CACHE = {}


def _build():
    if "nc" in _CACHE:
        return _CACHE["nc"]
    nc = bacc.Bacc("TRN2", target_bir_lowering=False, debug=False,
                   num_devices=NCORES)
    xt_dram = nc.declare_dram_parameter("xt", [128, XT_COLS], W_DT,
                                        isOutput=False)
    slab_dram = nc.declare_dram_parameter("slab", [SLAB_TOTAL], W_DT,
                                          isOutput=False)
    out_dram = nc.declare_dram_parameter("out", [128, OUTC],
                                         mybir.dt.bfloat16, isOutput=True)

    goffs = []
    off = 0
    for g in GROUPS:
        goffs.append(off)
        off += 128 * sum(WIDTHS[i] for i in g)
    assert off == SLAB_MAIN

    def slab_ap(off, n, p=128):
        return slab_dram[off:off + p * n].rearrange("(p n) -> p n", p=p)

    with ExitStack() as es:
        xt = es.enter_context(nc.sbuf_tensor("xt_sb", [128, XT_COLS], W_DT))
        wgs = [es.enter_context(
            nc.sbuf_tensor(f"wg{gi}", [128, sum(WIDTHS[i] for i in g)], W_DT))
            for gi, g in enumerate(GROUPS)]
        w32 = es.enter_context(nc.sbuf_tensor("w32_sb", [64, XCOLS], W_DT))
        obufs = [es.enter_context(
            nc.sbuf_tensor(f"ob{j}", [128, 512], mybir.dt.bfloat16))
            for j in range(6)]
        ox = es.enter_context(
            nc.sbuf_tensor("ox_sb", [128, XCOLS], mybir.dt.bfloat16))
        pss = [es.enter_context(
            nc.psum_tensor(f"ps{j}", [128, 512], mybir.dt.float32))
            for j in range(6)]
        psx = es.enter_context(
            nc.psum_tensor("psx", [128, XCOLS], mybir.dt.float32))
        psw = es.enter_context(
            nc.psum_tensor("psw", [128, 512], mybir.dt.float32))

        S_wa = nc.alloc_semaphore("S_wa")    # ring A (SP) input completions
        S_wb = nc.alloc_semaphore("S_wb")    # ring B (Act) input completions
        S_pe = nc.alloc_semaphore("S_pe")    # PE bank-finished milestones
        S_cp = nc.alloc_semaphore("S_cp")    # DVE drain copies done
        S_out = nc.alloc_semaphore("S_out")  # out DMA completions

        # ---- input DMA rings: A = SP issue queue, B = Activation's ----
        tokens = ["xtA", 0, 1, 2, 3, 4, 5, 6, 7, "w32", "xtB", 8, 9, 10,
                  11, 12]
        ring_a = tokens[0::2]
        ring_b = tokens[1::2]

        def issue_ring(eng, items, sem):
            thresh, v = {}, 0
            for gi in items:
                v += 16
                if gi == "xtA":
                    eng.dma_start(xt[:, :XT_A_COLS],
                                  xt_dram[:, :XT_A_COLS]).then_inc(sem, 16)
                elif gi == "xtB":
                    eng.dma_start(xt[:, XT_A_COLS:],
                                  xt_dram[:, XT_A_COLS:]).then_inc(sem, 16)
                elif gi == "w32":
                    eng.dma_start(
                        w32[:], slab_ap(SLAB_MAIN, XCOLS, p=64)
                    ).then_inc(sem, 16)
                else:
                    wsum = sum(WIDTHS[i] for i in GROUPS[gi])
                    eng.dma_start(
                        wgs[gi][:], slab_ap(goffs[gi], wsum)).then_inc(sem, 16)
                thresh[gi] = v
            return thresh

        wa_thresh = issue_ring(nc.sync, ring_a, S_wa)
        wb_thresh = issue_ring(nc.scalar, ring_b, S_wb)
        ring_of = {}
        for t in ring_a:
            ring_of[t] = (S_wa, wa_thresh[t])
        for t in ring_b:
            ring_of[t] = (S_wb, wb_thresh[t])

        # ---- PE: warmup, then positions descending in s ----
        pos_group = {}
        pos_off = {}
        for gi, g in enumerate(GROUPS):
            o = 0
            for i in g:
                pos_group[i] = gi
                pos_off[i] = o
                o += WIDTHS[i]

        def lhsT(i):
            # xt layout: [pos0..7, pos32, pos8..31]
            sl = i if i < NA else (NA if i == NPOS else i + 1)
            return xt[:, 128 * sl:128 * (sl + 1)]

        # (PE p-state warmup via junk matmuls was tried here and produced
        # sporadic NaN on some cores — reverted)

        for i in range(NPOS):
            gi = pos_group[i]
            if pos_off[i] == 0:              # first position of its group
                sem, v = ring_of[gi]
                nc.tensor.wait_ge(sem, v)
                if i == 0:
                    sem, v = ring_of["xtA"]
                    nc.tensor.wait_ge(sem, v)
                if i == 8:
                    sem, v = ring_of["xtB"]
                    nc.tensor.wait_ge(sem, v)
            w_i = WIDTHS[i]
            for j in range((w_i + 511) // 512):
                c0, c1 = 512 * j, min(512 * (j + 1), w_i)
                mm = nc.tensor.matmul(
                    pss[j][:, :c1 - c0],
                    lhsT(i),
                    wgs[gi][:, pos_off[i] + c0:pos_off[i] + c1],
                    start=(i == 0),
                    stop=(i == I_STOP[j]),
                )
                if i == I_STOP[j]:
                    mm.then_inc(S_pe, 1)
            if i == 8:                       # s=512 chunk, own bank
                sem, v = ring_of["w32"]
                nc.tensor.wait_ge(sem, v)
                nc.tensor.matmul(psx[:], xt[0:64, 128 * NA:128 * (NA + 1)],
                                 w32[0:64, :],
                                 start=True, stop=True).then_inc(S_pe, 1)

        # ---- DVE: drain copies in PE milestone order ----
        # milestones: pos5->bank5 (1), pos8->psx (2), pos10->bank4 (3),
        # pos15->bank3 (4), pos21->bank2 (5), pos26->bank1 (6), pos31->bank0 (7)
        drains = [(5, pss[5], obufs[5], 512, 2560),
                  (-1, psx, ox, XCOLS, NCOLS),
                  (4, pss[4], obufs[4], 512, 2048),
                  (3, pss[3], obufs[3], 512, 1536),
                  (2, pss[2], obufs[2], 512, 1024),
                  (1, pss[1], obufs[1], 512, 512),
                  (0, pss[0], obufs[0], 512, 0)]
        for n, (jb, ps, ob, cols, dst0) in enumerate(drains):
            nc.vector.wait_ge(S_pe, n + 1)
            nc.vector.tensor_copy(ob[:, :cols], ps[:, :cols]).then_inc(S_cp, 1)

        # ---- out DMAs: first five on the SP ring, last two (the latency-
        # critical tail) on the Activation ring, idle by then ----
        for n, (jb, ps, ob, cols, dst0) in enumerate(drains):
            eng = nc.sync if n < 5 else nc.scalar
            eng.wait_ge(S_cp, n + 1)
            eng.dma_start(out_dram[:, dst0:dst0 + cols],
                          ob[:, :cols]).then_inc(S_out, 16)
        nc.sync.wait_ge(S_out, 16 * len(drains))

    nc.compile()
    _CACHE["nc"] = nc
    return nc


def _quantize_weights(emb, W):
    """Greedy error-feedback e3m4 quantization of SCALE*W.

    Returns (xq8 (6,64) e3m4, Wq8 (513,513,6,64) e3m4 scaled, masked t<=s).
    Rounding of each 64-element w-row chooses floor/ceil per element to
    cancel the running residual r = A(q-w) + b0 where A = dequantized
    e3m4 embedding and b0 compensates the embedding's own quant error.
    """
    emb = np.asarray(emb, np.float32)
    W = np.asarray(W, np.float32)
    xq8 = emb.astype(NP_W)
    xq = xq8.astype(np.float32)                     # (6,64)
    ex = xq - emb

    Ws = W * SCALE                                  # (513,64,513,6) fp32
    B0 = np.tensordot(ex, Ws, axes=([1], [1]))      # (6,513,513,6)

    Wr = np.ascontiguousarray(Ws.transpose(0, 2, 3, 1)).reshape(-1, EMB)
    del Ws
    s_idx = np.repeat(np.arange(L1), L1 * V)
    t_idx = np.tile(np.repeat(np.arange(L1), V), L1)
    valid = t_idx <= s_idx
    Wv = np.ascontiguousarray(Wr[valid])            # (Nv, 64)
    r = np.ascontiguousarray(
        B0.transpose(1, 2, 3, 0).reshape(-1, V)[valid])
    del B0

    allb = np.arange(256, dtype=np.uint8)
    vals = allb.view(NP_W).astype(np.float32)
    grid = np.unique(vals[np.isfinite(vals)])
    lo_i = np.searchsorted(grid, Wv, side="right") - 1
    lo = grid[np.clip(lo_i, 0, len(grid) - 1)]
    hi = grid[np.clip(lo_i + 1, 0, len(grid) - 1)]
    del lo_i
    Q = np.empty_like(Wv)

    A = xq.T.copy()                                 # (64, 6)
    order = np.argsort(-np.linalg.norm(A, axis=1))
    for j in order:
        aj = A[j]
        n2 = float(aj @ aj)
        g = r @ aj
        dlo = lo[:, j] - Wv[:, j]
        dhi = hi[:, j] - Wv[:, j]
        pick_hi = 2 * g * dhi + dhi * dhi * n2 < 2 * g * dlo + dlo * dlo * n2
        d = np.where(pick_hi, dhi, dlo)
        Q[:, j] = np.where(pick_hi, hi[:, j], lo[:, j])
        r += d[:, None] * aj
    for j in order:                                 # one refinement sweep
        aj = A[j]
        n2 = float(aj @ aj)
        g = r @ aj
        cur = Q[:, j]
        other = np.where(cur == lo[:, j], hi[:, j], lo[:, j])
        dd = other - cur
        flip = 2 * g * dd + dd * dd * n2 < 0
        Q[:, j] = np.where(flip, other, cur)
        r += np.where(flip, dd, 0.0)[:, None] * aj

    Wq = np.zeros_like(Wr)
    Wq[valid] = Q
    Wq8 = Wq.reshape(L1, L1, V, EMB).astype(NP_W)   # (s,t,v,w)
    return xq8, Wq8


def _prep_inputs(src, embedding, weight):
    src = np.asarray(src)
    xq8, Wq8 = _quantize_weights(embedding, weight)

    xfull = xq8[src]                                # (B, 513, 64) e3m4
    row512 = np.ascontiguousarray(
        Wq8[512].transpose(2, 0, 1)).reshape(EMB, L1 * V)   # (64, 3078)

    in_maps = []
    for c in range(NCORES):
        ks = [8 * (NPOS - 1 - i) + c for i in range(NPOS)]
        s_arr = np.array([[2 * k, 2 * k + 1] for k in ks])   # (32,2)
        sel = xfull[:, s_arr, :]                     # (B,32,2,64)
        xtp = sel.transpose(2, 3, 1, 0).reshape(128, NPOS, B)
        xt = np.zeros((128, NPOS + 1, 128), NP_W)
        # slot layout: [pos0..7, pos32, pos8..31]
        xt[:, :NA, :] = xtp[:, :NA, :]
        xt[:EMB, NA, :] = xfull[:, 512, :].T
        xt[:, NA + 1:, :] = xtp[:, NA:, :]
        xt2 = np.ascontiguousarray(xt.reshape(128, XT_COLS))

        parts = []
        for g in GROUPS:
            blks = []
            for i in g:
                k = ks[i]
                t_hi = WIDTHS[i] // V
                arr = Wq8[2 * k:2 * k + 2, :t_hi, :, :]     # (2,t_hi,6,64)
                blks.append(arr.transpose(0, 3, 1, 2).reshape(128, WIDTHS[i]))
            parts.append(np.ascontiguousarray(
                np.concatenate(blks, axis=1)).reshape(-1))
        w32 = np.zeros((64, XCOLS), NP_W)
        c0 = XCOLS * c
        c1 = min(c0 + XCOLS, L1 * V)
        w32[:, :c1 - c0] = row512[:, c0:c1]
        parts.append(w32.reshape(-1))
        slab = np.concatenate(parts)
        assert slab.shape[0] == SLAB_TOTAL
        in_maps.append({"xt": xt2, "slab": slab})
    return in_maps


def _unshard(results, bias):
    full = np.zeros((B, L1 * V), np.float32)
    for c in range(NCORES):
        o = results[c]["out"].astype(np.float32)
        full[:, :NCOLS] += o[:, :NCOLS]
        c0 = XCOLS * c
        c1 = min(c0 + XCOLS, L1 * V)
        full[:, c0:c1] += o[:, NCOLS:NCOLS + (c1 - c0)]
    full *= 1.0 / SCALE
    full = full.reshape(B, L1, V) + np.asarray(bias, np.float32)[None]
    return np.ascontiguousarray(full.transpose(0, 2, 1))


def kernel(src, embedding, weight, bias):
    nc = _build()
    in_maps = _prep_inputs(src, embedding, weight)
    res = run_bass_kernel_spmd(nc, in_maps, list(range(NCORES)))
    return _unshard(res.results, bias)
